# revision 45
# baseline (speedup 1.0000x reference)
"""Trainium2 Bass kernel for nn_CRAU (per-channel sparse attention).

Computation (per batch b, channel c):
  qc  = Wq @ src (1x1 conv)
  S[c,t] = sum_d unfold(qc)[c,t,d] * feat[c,d] * (1/64)      t in 3x3 window
  A   = softmax_t(S);  vc = Wv @ feat + bv
  out = fold(A outer vc) * src

Sharding: 8 cores = 4 batches x 2 output-channel halves (no collectives).

Final schedule (v7), built from trace-driven iteration:
 - qc chunks are plane-aligned <=2048 cols; ONE shared PSUM pool
   [128,2048]x2 rotates q-conv chunks, the v-conv, and the fold tiles,
   so the PE is never gated by more than one evacuation.
 - qc PSUM->SBUF f16 copies all on Scalar (its only bulk duty besides
   fold/v-conv evacuations) so the copy stream tracks the DMA.
 - ALL NINE taps run as Vector TTR custom ops (product+reduce, no
   Scalar accumulates); the EO and OO taps are split into row-aligned
   chunk halves so they start the moment their qc rows land.
 - folds run on TensorE as diag(E_t) matmuls (diag built on Vector
   from an uploaded identity), PSUM f32, evacuated by Scalar to f16;
   Vector does one full-plane TT (F~ . src) per parity plane.
 - 1/sumE is applied AFTER (F~ . src) as 4x-mode tensor_scalars; the
   eo plane (computed post-softmax anyway) bakes r into its diags, and
   the last two planes' TT+DMA are split in halves to start the
   output DMA earlier.
 - deferred Scalar ops carry an explicit readiness chunk index so
   emission order can never invert a cross-engine dependency.
Plane layouts as v1/v2 (polyphase packed padded 129x129 grid).
Accumulator slot order: [t0,t2,t6,t8, t1,t7, t3,t5, t4].
"""

import numpy as np

N_CORES = 8
SCALE = 1.0 / 64.0

P_EE, P_EO, P_OE, P_OO = 0, 4290, 8450, 12674
SRCN = 16770
FEATN = 4290
OUTN = 16384

_prog_cache = {}
TRACE = False
TRACE_KW = {}
LAST_RESULT = [None]

# plane-aligned chunks: EE 4290 | EO 4160 | OE 4224 | OO 4096
CHUNKS = [2048, 2048, 194, 2048, 2048, 64, 2048, 2048, 128, 2048, 2048]
EE_RDY, EO_RDY, OE_RDY = 2, 5, 8
SLOT = {0: 0, 2: 1, 6: 2, 8: 3, 1: 4, 7: 5, 3: 6, 5: 7, 4: 8}
SLOT_ORDER = [0, 2, 6, 8, 1, 7, 3, 5, 4]


def _build(add_bv: bool, per_tap_bias: bool):
    import concourse.mybir as mybir
    import concourse.tile as tile
    from concourse import bacc
    from concourse.dve_ops import TENSOR_TENSOR_REDUCE

    f32 = mybir.dt.float32
    f16 = mybir.dt.float16
    ADD = mybir.AluOpType.add
    MULT = mybir.AluOpType.mult
    AX = mybir.AxisListType.X
    Exp = mybir.ActivationFunctionType.Exp
    Copy = mybir.ActivationFunctionType.Copy

    nc = bacc.Bacc("TRN2", target_bir_lowering=False, debug=False,
                   num_devices=N_CORES)

    src_d = nc.dram_tensor("src", [256, SRCN], f16, kind="ExternalInput").ap()
    feat_d = nc.dram_tensor("feat", [256, FEATN], f16,
                            kind="ExternalInput").ap()
    wpack_d = nc.dram_tensor("wpack", [256, 256], f16,
                             kind="ExternalInput").ap()
    sinit_d = nc.dram_tensor("s_init", [128, 9], f32,
                             kind="ExternalInput").ap()
    bv_d = nc.dram_tensor("bv", [128, 1], f32, kind="ExternalInput").ap()
    ident_d = nc.dram_tensor("ident", [128, 128], f16,
                             kind="ExternalInput").ap()
    out_d = nc.dram_tensor("out", [128, OUTN], f16, kind="ExternalOutput").ap()

    coff = [0]
    for cs in CHUNKS:
        coff.append(coff[-1] + cs)

    with tile.TileContext(nc) as tc:
        with (
            tc.tile_pool(name="constp", bufs=2) as constp,
            tc.tile_pool(name="srcp", bufs=2) as srcp,
            tc.tile_pool(name="featp", bufs=2) as featp,
            tc.tile_pool(name="qcp", bufs=1) as qcp,
            tc.tile_pool(name="vcp", bufs=1) as vcp,
            tc.tile_pool(name="smp", bufs=1) as smp,
            tc.tile_pool(name="prodp", bufs=4) as prodp,
            tc.tile_pool(name="outp", bufs=4) as outp,
            tc.tile_pool(name="ps", bufs=2, space="PSUM") as ps,
        ):
            # smalls: [0:9] S by slot [9:18] E [52:60] EE halves [27] sumE
            # [28] r [32:41] s_init [48] bv
            sm = smp.tile([128, 64], f32, tag="smalls")
            nc.sync.dma_start(sm[:, 32:41], sinit_d[:, :])
            if add_bv:
                nc.sync.dma_start(sm[:, 48:49], bv_d[:, :])

            w_t = []
            for kt in range(2):
                wt = constp.tile([128, 256], f16, tag="w")
                nc.sync.dma_start(wt[:], wpack_d[128 * kt:128 * kt + 128, :])
                w_t.append(wt)
            ident = constp.tile([128, 128], f16, tag="ident")
            nc.sync.dma_start(ident[:], ident_d[:, :])

            feat_t = [featp.tile([128, FEATN], f16, tag="feat",
                                 name=f"feat{k}") for k in range(2)]

            src_t = [srcp.tile([128, SRCN], f16, tag="src", name=f"src{k}")
                     for k in range(2)]
            for c in range(len(CHUNKS)):
                if c == 3:
                    nc.sync.dma_start(feat_t[0][:], feat_d[0:128, :])
                if c == 5:
                    nc.sync.dma_start(feat_t[1][:], feat_d[128:256, :])
                for kt in range(2):
                    nc.sync.dma_start(
                        src_t[kt][:, coff[c]:coff[c + 1]],
                        src_d[128 * kt:128 * kt + 128, coff[c]:coff[c + 1]])

            qc = qcp.tile([128, SRCN], f16, tag="qc")
            qEE = qc[:, P_EE:P_EO].rearrange("p (r q) -> p r q", q=66)
            qEO = qc[:, P_EO:P_OE].rearrange("p (r q) -> p r q", q=64)
            qOE = qc[:, P_OE:P_OO].rearrange("p (r q) -> p r q", q=66)
            qOO = qc[:, P_OO:SRCN].rearrange("p (r q) -> p r q", q=64)
            kv = feat_t[0].rearrange("p (r q) -> p r q", q=66)[:, 0:64, 0:64]

            pr = [prodp.tile([128, 4096], f16, tag="prod", name=f"pr{k}")
                  for k in range(4)]
            dg = constp.tile([128, 9 * 128], f16, tag="diag")

            pend_s = []            # deferred (ready_chunk, scalar-op) items

            def pend_pop(c):
                done = [e for e in pend_s if e[0] <= c]
                for e in done:
                    e[1]()
                    pend_s.remove(e)

            def tap_ttr(t, qview, scr, sl=None, kview=None):
                sl = SLOT[t] if sl is None else sl
                kview = kv if kview is None else kview
                n = 1
                for d in qview.shape[1:]:
                    n *= d
                nc.vector._custom_dve(
                    TENSOR_TENSOR_REDUCE,
                    out=scr[:, 0:n].rearrange(
                        "p (r q) -> p r q", q=qview.shape[-1]),
                    in0=qview, in1=kview, s0=0.0,
                    s1=SCALE, accum_out=sm[:, sl:sl + 1])

            def tap_tth(qview, kview, scr, off, sl, ready):
                n = qview.shape[1] * qview.shape[2]
                s3 = scr[:, off:off + n].rearrange(
                    "p (r q) -> p r q", q=qview.shape[-1])
                nc.vector.tensor_tensor(out=s3, in0=qview, in1=kview,
                                        op=MULT)

                def acc(scr=scr, off=off, n=n, sl=sl):
                    nc.scalar.activation(scr[:, off:off + n],
                                         scr[:, off:off + n],
                                         Copy, bias=0.0, scale=SCALE,
                                         accum_out=sm[:, sl:sl + 1])
                pend_s.append((ready, acc))

            def exp_group(sl0, sl1):
                if per_tap_bias:
                    for sl in range(sl0, sl1):
                        nc.scalar.activation(sm[:, 9 + sl:10 + sl],
                                             sm[:, sl:sl + 1], Exp,
                                             bias=sm[:, 32 + sl:33 + sl],
                                             scale=1.0)
                else:
                    nc.scalar.activation(sm[:, 9 + sl0:9 + sl1],
                                         sm[:, sl0:sl1], Exp,
                                         bias=0.0, scale=1.0)

            def diag(sl, rmul=False):
                if rmul:
                    nc.vector.tensor_scalar(
                        out=dg[:, sl * 128:sl * 128 + 128], in0=ident[:],
                        scalar1=sm[:, 9 + sl:10 + sl],
                        scalar2=sm[:, 28:29], op0=MULT, op1=MULT)
                else:
                    nc.vector.tensor_scalar(
                        out=dg[:, sl * 128:sl * 128 + 128], in0=ident[:],
                        scalar1=sm[:, 9 + sl:10 + sl], scalar2=None,
                        op0=MULT)

            def emit_vconv():
                vc = vcp.tile([128, FEATN], f16, tag="vc")
                for c0 in (0, 2048, 4096):
                    csz = min(2048, FEATN - c0)
                    pt = ps.tile([128, 2048], f32, tag="mm")
                    for kt in range(2):
                        for s0 in range(0, csz, 512):
                            ssz = min(512, csz - s0)
                            nc.tensor.matmul(
                                pt[:, s0:s0 + ssz],
                                lhsT=w_t[kt][:, 128:256],
                                rhs=feat_t[kt][:, c0 + s0:c0 + s0 + ssz],
                                start=(kt == 0), stop=(kt == 1))
                    if add_bv:
                        nc.vector.tensor_scalar(
                            out=vc[:, c0:c0 + csz], in0=pt[:, 0:csz],
                            scalar1=sm[:, 48:49], scalar2=None, op0=ADD)
                    else:
                        nc.scalar.copy(vc[:, c0:c0 + csz], pt[:, 0:csz])
                vc3 = vc.rearrange("p (r q) -> p r q", q=66)
                if add_bv:
                    nc.gpsimd.memset(vc3[:, 64, :], 0.0)
                    nc.gpsimd.memset(vc3[:, :, 64:66], 0.0)
                return vc3

            vc3 = None
            views = {}

            s3 = src_t[0]
            sEE = s3[:, P_EE:P_EO].rearrange("p (r q) -> p r q", q=66)
            sEO = s3[:, P_EO:P_OE].rearrange("p (r q) -> p r q", q=64)
            sOE = s3[:, P_OE:P_OO].rearrange("p (r q) -> p r q", q=66)
            sOO = s3[:, P_OO:SRCN].rearrange("p (r q) -> p r q", q=64)

            oEE = outp.tile([128, 4096], f16, tag="O", name="oEE")
            oEO = outp.tile([128, 4096], f16, tag="O", name="oEO")
            oOE = outp.tile([128, 4096], f16, tag="O", name="oOE")
            oOO = outp.tile([128, 4096], f16, tag="O", name="oOO")

            fscr = pr[2]       # t8's product is consumed by chunk 7

            def fold_half(slots, vkeys, h, out_tile=None, srcv3=None):
                # diag matmuls, PSUM-accumulated 2048-col half. If
                # out_tile given: Vector TT multiplies PSUM F~ by the src
                # half directly; else returns PSUM tile for Scalar evac.
                ft = ps.tile([128, 2048], f32, tag="mm", name="ft")
                f3 = ft.rearrange("p (r q) -> p r q", q=64)
                for i, (sl, vk) in enumerate(zip(slots, vkeys)):
                    v3 = views[vk]
                    for b in range(4):
                        r0 = h * 32 + b * 8
                        nc.tensor.matmul(
                            f3[:, b * 8:b * 8 + 8, :],
                            lhsT=dg[:, sl * 128:sl * 128 + 128],
                            rhs=v3[:, r0:r0 + 8, :],
                            start=(i == 0), stop=(i == len(slots) - 1))
                if out_tile is None:
                    return ft
                nc.vector.tensor_tensor(
                    out=out_tile[:, h * 2048:h * 2048 + 2048]
                        .rearrange("p (r q) -> p r q", q=64),
                    in0=f3[:], in1=srcv3[:, h * 32:h * 32 + 32, :],
                    op=MULT)

            for c, csz in enumerate(CHUNKS):
                c0 = coff[c]
                if c == 7:
                    vc3 = emit_vconv()
                    views = dict(v00=vc3[:, 0:64, 0:64],
                                 v10=vc3[:, 1:65, 0:64],
                                 v01=vc3[:, 0:64, 1:65],
                                 v11=vc3[:, 1:65, 1:65])
                pt = ps.tile([128, 2048], f32, tag="mm")
                for kt in range(2):
                    for s0 in range(0, csz, 512):
                        ssz = min(512, csz - s0)
                        nc.tensor.matmul(
                            pt[:, s0:s0 + ssz],
                            lhsT=w_t[kt][:, 0:128],
                            rhs=src_t[kt][:, c0 + s0:c0 + s0 + ssz],
                            start=(kt == 0), stop=(kt == 1))
                nc.scalar.copy(qc[:, c0:c0 + csz], pt[:, 0:csz])
                pend_pop(c)

                if c == 0:
                    # EE tap first halves (rows fully inside chunk 0)
                    tap_ttr(0, qEE[:, 0:31, 1:65], pr[3], sl=52,
                            kview=kv[:, 0:31, :])
                    tap_tth(qEE[:, 0:31, 2:66], kv[:, 0:31, :],
                            pr[0], 0, 53, 2)
                    tap_tth(qEE[:, 1:31, 1:65], kv[:, 0:30, :],
                            pr[1], 0, 54, 2)
                    tap_ttr(8, qEE[:, 1:31, 2:66], pr[3], sl=55,
                            kview=kv[:, 0:30, :])
                if c == EE_RDY:
                    tap_ttr(0, qEE[:, 31:64, 1:65], pr[3], sl=56,
                            kview=kv[:, 31:64, :])
                    tap_tth(qEE[:, 31:64, 2:66], kv[:, 31:64, :],
                            pr[0], 1984, 57, 3)
                    tap_tth(qEE[:, 31:65, 1:65], kv[:, 30:64, :],
                            pr[1], 1920, 58, 4)
                    tap_ttr(8, qEE[:, 31:65, 2:66], pr[3], sl=59,
                            kview=kv[:, 30:64, :])

                    def fin_ee():
                        exp_group(0, 4)
                    pend_s.append((6, fin_ee))
                if c == 4:
                    tap_ttr(1, qEO[:, 0:64, 0:64], pr[3])
                if c == EO_RDY:
                    # EE half-sums (all 8 half accumulators written by now)
                    for i in range(4):
                        nc.vector.tensor_tensor(
                            out=sm[:, i:i + 1], in0=sm[:, 52 + i:53 + i],
                            in1=sm[:, 56 + i:57 + i], op=ADD)
                    tap_ttr(7, qEO[:, 1:65, 0:64], pr[3])

                    def fin_eo():
                        exp_group(4, 6)
                    pend_s.append((6, fin_eo))
                if c == OE_RDY:
                    for sl in range(6):
                        diag(sl)
                    tap_ttr(3, qOE[:, 0:64, 1:65], pr[3])
                    tap_ttr(5, qOE[:, 0:64, 2:66], pr[3])
                if c == 10:
                    tap_ttr(4, qOO[:, 0:64, 0:64], pr[3])

            pend_pop(99)
            # folds after the last q-conv chunk so the late qc copies are
            # never queued behind fold matmuls / diag dependencies
            for h in range(2):
                ftOO = fold_half([0, 1, 2, 3],
                                 ['v11', 'v10', 'v01', 'v00'], h)
                nc.scalar.copy(fscr[:, h * 2048:h * 2048 + 2048], ftOO[:])
            for h in range(2):
                ftOE = fold_half([4, 5], ['v10', 'v00'], h)
                nc.scalar.copy(pr[1][:, h * 2048:h * 2048 + 2048], ftOE[:])
            exp_group(6, 9)

            # ---- normalization ----
            nc.vector.tensor_reduce(sm[:, 27:28], sm[:, 9:18],
                                    axis=AX, op=ADD)
            nc.vector.reciprocal(sm[:, 28:29], sm[:, 27:28])
            r = sm[:, 28:29]

            # eo fold runs post-r: bake r into its diags (A = E*r)
            diag(6, rmul=True)
            diag(7, rmul=True)
            for h in range(2):
                ftEO = fold_half([6, 7], ['v01', 'v00'], h)
                nc.scalar.copy(pr[0][:, h * 2048:h * 2048 + 2048], ftEO[:])

            # ---- tail: finish planes in readiness order ----
            nc.vector.tensor_tensor(
                out=oOO.rearrange("p (r q) -> p r q", q=64),
                in0=fscr.rearrange("p (r q) -> p r q", q=64),
                in1=sEE[:, 1:65, 2:66], op=MULT)
            nc.vector.tensor_scalar(out=oOO[:], in0=oOO[:], scalar1=r,
                                    scalar2=None, op0=MULT)
            nc.sync.dma_start(out_d[:, 12288:16384], oOO[:])
            nc.vector.tensor_tensor(
                out=oOE.rearrange("p (r q) -> p r q", q=64),
                in0=pr[1].rearrange("p (r q) -> p r q", q=64),
                in1=sEO[:, 1:65, 0:64], op=MULT)
            nc.vector.tensor_scalar(out=oOE[:], in0=oOE[:], scalar1=r,
                                    scalar2=None, op0=MULT)
            nc.sync.dma_start(out_d[:, 8192:12288], oOE[:])
            # ee: (E4*r*v00) . sOO
            nc.vector.tensor_scalar(
                out=pr[3][:, 0:4096].rearrange("p (r q) -> p r q", q=64),
                in0=views['v00'], scalar1=sm[:, 17:18], scalar2=r,
                op0=MULT, op1=MULT)
            for h in range(2):
                nc.vector.tensor_tensor(
                    out=oEE[:, h * 2048:h * 2048 + 2048]
                        .rearrange("p (r q) -> p r q", q=64),
                    in0=pr[3][:, h * 2048:h * 2048 + 2048]
                        .rearrange("p (r q) -> p r q", q=64),
                    in1=sOO[:, h * 32:h * 32 + 32, 0:64], op=MULT)
                nc.sync.dma_start(out_d[:, h * 2048:h * 2048 + 2048],
                                  oEE[:, h * 2048:h * 2048 + 2048])
            for h in range(2):
                nc.vector.tensor_tensor(
                    out=oEO[:, h * 2048:h * 2048 + 2048]
                        .rearrange("p (r q) -> p r q", q=64),
                    in0=pr[0][:, h * 2048:h * 2048 + 2048]
                        .rearrange("p (r q) -> p r q", q=64),
                    in1=sOE[:, h * 32:h * 32 + 32, 2:66], op=MULT)
                nc.sync.dma_start(out_d[:, 4096 + h * 2048:4096 + (h + 1) * 2048],
                                  oEO[:, h * 2048:h * 2048 + 2048])

    nc.compile()
    return nc


def _get_program(add_bv: bool, per_tap_bias: bool):
    key = (add_bv, per_tap_bias)
    if key not in _prog_cache:
        _prog_cache[key] = _build(add_bv, per_tap_bias)
    return _prog_cache[key]


def _polyphase(x):
    B, C = x.shape[:2]
    ee = np.zeros((B, C, 65, 66), np.float16)
    ee[:, :, :, 1:66] = x[:, :, 0::2, 0::2]
    oe = np.zeros((B, C, 64, 66), np.float16)
    oe[:, :, :, 1:66] = x[:, :, 1::2, 0::2]
    return np.concatenate([
        ee.reshape(B, C, -1),
        x[:, :, 0::2, 1::2].reshape(B, C, -1),
        oe.reshape(B, C, -1),
        x[:, :, 1::2, 1::2].reshape(B, C, -1),
    ], axis=2)


def kernel(feat, src, Wq, bq, Wv, bv):
    from concourse.bass_utils import run_bass_kernel_spmd

    feat = np.asarray(feat, dtype=np.float32)
    src = np.asarray(src, dtype=np.float32)
    Wq = np.asarray(Wq, dtype=np.float32)
    bq = np.asarray(bq, dtype=np.float32)
    Wv = np.asarray(Wv, dtype=np.float32)
    bv = np.asarray(bv, dtype=np.float32)
    B, C, H, W = src.shape
    CH_HALF = C // 2

    src_pad = np.zeros((B, C, 129, 129), np.float16)
    src_pad[:, :, 1:129, 1:129] = src
    src_pk = _polyphase(src_pad)
    feat_pk = np.zeros((B, C, 65, 66), np.float16)
    feat_pk[:, :, 0:64, 0:64] = feat
    feat_pk = feat_pk.reshape(B, C, FEATN)

    add_bv = bool(np.any(bv))
    per_tap_bias = bool(np.any(bq))
    nc = _get_program(add_bv, per_tap_bias)
    ident = np.eye(128, dtype=np.float16)

    in_maps = []
    for core in range(N_CORES):
        b, u = core // 2, core % 2
        own = slice(CH_HALF * u, CH_HALF * u + CH_HALF)
        perm = np.r_[own, slice(CH_HALF * (1 - u), CH_HALF * (1 - u) + CH_HALF)]
        wpack = np.concatenate(
            [Wq[own][:, perm].T, Wv[own][:, perm].T], axis=1
        ).astype(np.float16)
        if per_tap_bias:
            k = feat[b, own].astype(np.float64)
            tot = k.sum((1, 2))
            no_r0 = tot - k[:, 0, :].sum(1)
            no_c0 = tot - k[:, :, 0].sum(1)
            no_rc = no_r0 - k[:, :, 0].sum(1) + k[:, 0, 0]
            sums = [no_rc, no_r0, no_r0, no_c0, tot, tot, no_c0, tot, tot]
            sinit_t = (np.stack(sums, 1) * bq[own, None] * SCALE).astype(
                np.float32)
            sinit = sinit_t[:, SLOT_ORDER]
        else:
            sinit = np.zeros((CH_HALF, 9), np.float32)
        in_maps.append({
            "src": np.ascontiguousarray(src_pk[b, perm]),
            "feat": np.ascontiguousarray(feat_pk[b, perm]),
            "wpack": np.ascontiguousarray(wpack),
            "s_init": sinit,
            "bv": bv[own].reshape(CH_HALF, 1).astype(np.float32),
            "ident": ident,
        })

    res = run_bass_kernel_spmd(nc, in_maps, list(range(N_CORES)),
                               trace=TRACE, **TRACE_KW)
    LAST_RESULT[0] = res

    out = np.empty((B, C, H, W), np.float32)
    for core in range(N_CORES):
        b, u = core // 2, core % 2
        own = slice(CH_HALF * u, CH_HALF * u + CH_HALF)
        r = res.results[core]["out"].astype(np.float32).reshape(
            CH_HALF, 4, 64, 64)
        out[b, own, 0::2, 0::2] = r[:, 0]
        out[b, own, 0::2, 1::2] = r[:, 1]
        out[b, own, 1::2, 0::2] = r[:, 2]
        out[b, own, 1::2, 1::2] = r[:, 3]
    return out


# revision 46
# speedup vs baseline: 1.0988x; 1.0988x over previous
"""Trainium2 Bass kernel for nn_CRAU (per-channel sparse attention).

Computation (per batch b, channel c):
  qc  = Wq @ src (1x1 conv)
  S[c,t] = sum_d unfold(qc)[c,t,d] * feat[c,d] * (1/64)      t in 3x3 window
  A   = softmax_t(S);  vc = Wv @ feat + bv
  out = fold(A outer vc) * src

Sharding: 8 cores = 4 batches x 2 output-channel halves (no collectives).

Final schedule (v7), built from trace-driven iteration:
 - qc chunks are plane-aligned <=2048 cols; ONE shared PSUM pool
   [128,2048]x2 rotates q-conv chunks, the v-conv, and the fold tiles,
   so the PE is never gated by more than one evacuation.
 - qc PSUM->SBUF f16 copies all on Scalar (its only bulk duty besides
   fold/v-conv evacuations) so the copy stream tracks the DMA.
 - ALL NINE taps run as Vector TTR custom ops (product+reduce, no
   Scalar accumulates); the EO and OO taps are split into row-aligned
   chunk halves so they start the moment their qc rows land.
 - folds run on TensorE as diag(E_t) matmuls (diag built on Vector
   from an uploaded identity), PSUM f32, evacuated by Scalar to f16;
   Vector does one full-plane TT (F~ . src) per parity plane.
 - 1/sumE is applied AFTER (F~ . src) as 4x-mode tensor_scalars; the
   eo plane (computed post-softmax anyway) bakes r into its diags, and
   the last two planes' TT+DMA are split in halves to start the
   output DMA earlier.
 - deferred Scalar ops carry an explicit readiness chunk index so
   emission order can never invert a cross-engine dependency.
Plane layouts as v1/v2 (polyphase packed padded 129x129 grid).
Accumulator slot order: [t0,t2,t6,t8, t1,t7, t3,t5, t4].
"""

import numpy as np

N_CORES = 8
SCALE = 1.0 / 64.0

P_EE, P_EO, P_OE, P_OO = 0, 4290, 8450, 12674
SRCN = 16770
FEATN = 4290
OUTN = 16384

_prog_cache = {}
TRACE = False
TRACE_KW = {}
LAST_RESULT = [None]

# plane-aligned chunks: EE 4290 | EO 4160 | OE 4224 | OO 4096
CHUNKS = [2048, 2048, 194, 2048, 2048, 64, 2048, 2048, 128, 2048, 2048]
EE_RDY, EO_RDY, OE_RDY = 2, 5, 8
SLOT = {0: 0, 2: 1, 6: 2, 8: 3, 1: 4, 7: 5, 3: 6, 5: 7, 4: 8}
SLOT_ORDER = [0, 2, 6, 8, 1, 7, 3, 5, 4]


def _build(add_bv: bool, per_tap_bias: bool):
    import concourse.mybir as mybir
    import concourse.tile as tile
    from concourse import bacc
    from concourse.dve_ops import TENSOR_TENSOR_REDUCE

    f32 = mybir.dt.float32
    f16 = mybir.dt.float16
    ADD = mybir.AluOpType.add
    MULT = mybir.AluOpType.mult
    AX = mybir.AxisListType.X
    Exp = mybir.ActivationFunctionType.Exp
    Copy = mybir.ActivationFunctionType.Copy

    nc = bacc.Bacc("TRN2", target_bir_lowering=False, debug=False,
                   num_devices=N_CORES)

    src_d = nc.dram_tensor("src", [256, SRCN], f16, kind="ExternalInput").ap()
    feat_d = nc.dram_tensor("feat", [256, FEATN], f16,
                            kind="ExternalInput").ap()
    wpack_d = nc.dram_tensor("wpack", [256, 256], f16,
                             kind="ExternalInput").ap()
    sinit_d = nc.dram_tensor("s_init", [128, 9], f32,
                             kind="ExternalInput").ap()
    bv_d = nc.dram_tensor("bv", [128, 1], f32, kind="ExternalInput").ap()
    ident_d = nc.dram_tensor("ident", [128, 128], f16,
                             kind="ExternalInput").ap()
    out_d = nc.dram_tensor("out", [128, OUTN], f16, kind="ExternalOutput").ap()

    coff = [0]
    for cs in CHUNKS:
        coff.append(coff[-1] + cs)

    with tile.TileContext(nc) as tc:
        with (
            tc.tile_pool(name="constp", bufs=2) as constp,
            tc.tile_pool(name="srcp", bufs=2) as srcp,
            tc.tile_pool(name="featp", bufs=2) as featp,
            tc.tile_pool(name="qcp", bufs=1) as qcp,
            tc.tile_pool(name="vcp", bufs=1) as vcp,
            tc.tile_pool(name="smp", bufs=1) as smp,
            tc.tile_pool(name="prodp", bufs=4) as prodp,
            tc.tile_pool(name="outp", bufs=4) as outp,
            tc.tile_pool(name="ps", bufs=2, space="PSUM") as ps,
        ):
            # smalls: [0:9] S by slot [9:18] E [52:60] EE halves [27] sumE
            # [28] r [32:41] s_init [48] bv
            sm = smp.tile([128, 64], f32, tag="smalls")
            nc.sync.dma_start(sm[:, 32:41], sinit_d[:, :])
            if add_bv:
                nc.sync.dma_start(sm[:, 48:49], bv_d[:, :])

            w_t = []
            for kt in range(2):
                wt = constp.tile([128, 256], f16, tag="w")
                nc.sync.dma_start(wt[:], wpack_d[128 * kt:128 * kt + 128, :])
                w_t.append(wt)
            ident = constp.tile([128, 128], f16, tag="ident")
            nc.sync.dma_start(ident[:], ident_d[:, :])

            feat_t = [featp.tile([128, FEATN], f16, tag="feat",
                                 name=f"feat{k}") for k in range(2)]

            src_t = [srcp.tile([128, SRCN], f16, tag="src", name=f"src{k}")
                     for k in range(2)]
            for c in range(len(CHUNKS)):
                if c == 3:
                    nc.sync.dma_start(feat_t[0][:], feat_d[0:128, :])
                if c == 5:
                    nc.sync.dma_start(feat_t[1][:], feat_d[128:256, :])
                for kt in range(2):
                    nc.sync.dma_start(
                        src_t[kt][:, coff[c]:coff[c + 1]],
                        src_d[128 * kt:128 * kt + 128, coff[c]:coff[c + 1]])

            qc = qcp.tile([128, SRCN], f16, tag="qc")
            qEE = qc[:, P_EE:P_EO].rearrange("p (r q) -> p r q", q=66)
            qEO = qc[:, P_EO:P_OE].rearrange("p (r q) -> p r q", q=64)
            qOE = qc[:, P_OE:P_OO].rearrange("p (r q) -> p r q", q=66)
            qOO = qc[:, P_OO:SRCN].rearrange("p (r q) -> p r q", q=64)
            kv = feat_t[0].rearrange("p (r q) -> p r q", q=66)[:, 0:64, 0:64]

            pr = [prodp.tile([128, 4096], f16, tag="prod", name=f"pr{k}")
                  for k in range(4)]
            dg = constp.tile([128, 9 * 128], f16, tag="diag")

            pend_s = []            # deferred (ready_chunk, scalar-op) items

            def pend_pop(c):
                done = [e for e in pend_s if e[0] <= c]
                for e in done:
                    e[1]()
                    pend_s.remove(e)

            def tap_ttr(t, qview, scr, sl=None, kview=None):
                sl = SLOT[t] if sl is None else sl
                kview = kv if kview is None else kview
                n = 1
                for d in qview.shape[1:]:
                    n *= d
                nc.vector._custom_dve(
                    TENSOR_TENSOR_REDUCE,
                    out=scr[:, 0:n].rearrange(
                        "p (r q) -> p r q", q=qview.shape[-1]),
                    in0=qview, in1=kview, s0=0.0,
                    s1=SCALE, accum_out=sm[:, sl:sl + 1])

            def tap_tth(qview, kview, scr, off, sl, ready):
                n = qview.shape[1] * qview.shape[2]
                s3 = scr[:, off:off + n].rearrange(
                    "p (r q) -> p r q", q=qview.shape[-1])
                nc.vector.tensor_tensor(out=s3, in0=qview, in1=kview,
                                        op=MULT)

                def acc(scr=scr, off=off, n=n, sl=sl):
                    nc.scalar.activation(scr[:, off:off + n],
                                         scr[:, off:off + n],
                                         Copy, bias=0.0, scale=SCALE,
                                         accum_out=sm[:, sl:sl + 1])
                pend_s.append((ready, acc))

            def exp_group(sl0, sl1):
                if per_tap_bias:
                    for sl in range(sl0, sl1):
                        nc.scalar.activation(sm[:, 9 + sl:10 + sl],
                                             sm[:, sl:sl + 1], Exp,
                                             bias=sm[:, 32 + sl:33 + sl],
                                             scale=1.0)
                else:
                    nc.scalar.activation(sm[:, 9 + sl0:9 + sl1],
                                         sm[:, sl0:sl1], Exp,
                                         bias=0.0, scale=1.0)

            def diag(sl, rmul=False):
                if rmul:
                    nc.vector.tensor_scalar(
                        out=dg[:, sl * 128:sl * 128 + 128], in0=ident[:],
                        scalar1=sm[:, 9 + sl:10 + sl],
                        scalar2=sm[:, 28:29], op0=MULT, op1=MULT)
                else:
                    nc.vector.tensor_scalar(
                        out=dg[:, sl * 128:sl * 128 + 128], in0=ident[:],
                        scalar1=sm[:, 9 + sl:10 + sl], scalar2=None,
                        op0=MULT)

            def emit_vconv():
                vc = vcp.tile([128, FEATN], f16, tag="vc")
                for c0 in (0, 2048, 4096):
                    csz = min(2048, FEATN - c0)
                    pt = ps.tile([128, 2048], f32, tag="mm")
                    for kt in range(2):
                        for s0 in range(0, csz, 512):
                            ssz = min(512, csz - s0)
                            nc.tensor.matmul(
                                pt[:, s0:s0 + ssz],
                                lhsT=w_t[kt][:, 128:256],
                                rhs=feat_t[kt][:, c0 + s0:c0 + s0 + ssz],
                                start=(kt == 0), stop=(kt == 1))
                    if add_bv:
                        nc.vector.tensor_scalar(
                            out=vc[:, c0:c0 + csz], in0=pt[:, 0:csz],
                            scalar1=sm[:, 48:49], scalar2=None, op0=ADD)
                    else:
                        nc.scalar.copy(vc[:, c0:c0 + csz], pt[:, 0:csz])
                vc3 = vc.rearrange("p (r q) -> p r q", q=66)
                if add_bv:
                    nc.gpsimd.memset(vc3[:, 64, :], 0.0)
                    nc.gpsimd.memset(vc3[:, :, 64:66], 0.0)
                return vc3

            vc3 = None
            views = {}

            s3 = src_t[0]
            sEE = s3[:, P_EE:P_EO].rearrange("p (r q) -> p r q", q=66)
            sEO = s3[:, P_EO:P_OE].rearrange("p (r q) -> p r q", q=64)
            sOE = s3[:, P_OE:P_OO].rearrange("p (r q) -> p r q", q=66)
            sOO = s3[:, P_OO:SRCN].rearrange("p (r q) -> p r q", q=64)

            oEE = outp.tile([128, 4096], f16, tag="O", name="oEE")
            oEO = outp.tile([128, 4096], f16, tag="O", name="oEO")
            oOE = outp.tile([128, 4096], f16, tag="O", name="oOE")
            oOO = outp.tile([128, 4096], f16, tag="O", name="oOO")

            fscr = pr[2]       # t8's product is consumed by chunk 7

            def fold_half(slots, vkeys, h, out_tile=None, srcv3=None):
                # diag matmuls, PSUM-accumulated 2048-col half. If
                # out_tile given: Vector TT multiplies PSUM F~ by the src
                # half directly; else returns PSUM tile for Scalar evac.
                ft = ps.tile([128, 2048], f32, tag="mm", name="ft")
                f3 = ft.rearrange("p (r q) -> p r q", q=64)
                for i, (sl, vk) in enumerate(zip(slots, vkeys)):
                    v3 = views[vk]
                    for b in range(4):
                        r0 = h * 32 + b * 8
                        nc.tensor.matmul(
                            f3[:, b * 8:b * 8 + 8, :],
                            lhsT=dg[:, sl * 128:sl * 128 + 128],
                            rhs=v3[:, r0:r0 + 8, :],
                            start=(i == 0), stop=(i == len(slots) - 1))
                if out_tile is None:
                    return ft
                nc.vector.tensor_tensor(
                    out=out_tile[:, h * 2048:h * 2048 + 2048]
                        .rearrange("p (r q) -> p r q", q=64),
                    in0=f3[:], in1=srcv3[:, h * 32:h * 32 + 32, :],
                    op=MULT)

            for c, csz in enumerate(CHUNKS):
                c0 = coff[c]
                if c == 7:
                    vc3 = emit_vconv()
                    views = dict(v00=vc3[:, 0:64, 0:64],
                                 v10=vc3[:, 1:65, 0:64],
                                 v01=vc3[:, 0:64, 1:65],
                                 v11=vc3[:, 1:65, 1:65])
                pt = ps.tile([128, 2048], f32, tag="mm")
                for kt in range(2):
                    for s0 in range(0, csz, 512):
                        ssz = min(512, csz - s0)
                        nc.tensor.matmul(
                            pt[:, s0:s0 + ssz],
                            lhsT=w_t[kt][:, 0:128],
                            rhs=src_t[kt][:, c0 + s0:c0 + s0 + ssz],
                            start=(kt == 0), stop=(kt == 1))
                nc.scalar.copy(qc[:, c0:c0 + csz], pt[:, 0:csz])
                pend_pop(c)

                if c == 0:
                    # EE tap first halves (rows fully inside chunk 0)
                    tap_ttr(0, qEE[:, 0:31, 1:65], pr[3], sl=52,
                            kview=kv[:, 0:31, :])
                    tap_tth(qEE[:, 0:31, 2:66], kv[:, 0:31, :],
                            pr[0], 0, 53, 2)
                    tap_tth(qEE[:, 1:31, 1:65], kv[:, 0:30, :],
                            pr[1], 0, 54, 2)
                    tap_ttr(8, qEE[:, 1:31, 2:66], pr[3], sl=55,
                            kview=kv[:, 0:30, :])
                if c == EE_RDY:
                    tap_ttr(0, qEE[:, 31:64, 1:65], pr[3], sl=56,
                            kview=kv[:, 31:64, :])
                    tap_tth(qEE[:, 31:64, 2:66], kv[:, 31:64, :],
                            pr[0], 1984, 57, 3)
                    tap_tth(qEE[:, 31:65, 1:65], kv[:, 30:64, :],
                            pr[1], 1920, 58, 4)
                    tap_ttr(8, qEE[:, 31:65, 2:66], pr[3], sl=59,
                            kview=kv[:, 30:64, :])

                    def fin_ee():
                        exp_group(0, 4)
                    pend_s.append((6, fin_ee))
                if c == 4:
                    tap_ttr(1, qEO[:, 0:64, 0:64], pr[3])
                if c == EO_RDY:
                    # EE half-sums (all 8 half accumulators written by now)
                    for i in range(4):
                        nc.vector.tensor_tensor(
                            out=sm[:, i:i + 1], in0=sm[:, 52 + i:53 + i],
                            in1=sm[:, 56 + i:57 + i], op=ADD)
                    tap_ttr(7, qEO[:, 1:65, 0:64], pr[3])

                    def fin_eo():
                        exp_group(4, 6)
                    pend_s.append((6, fin_eo))
                if c == OE_RDY:
                    for sl in range(6):
                        diag(sl)
                    tap_ttr(3, qOE[:, 0:64, 1:65], pr[3])
                    tap_ttr(5, qOE[:, 0:64, 2:66], pr[3])
                if c == 10:
                    tap_ttr(4, qOO[:, 0:64, 0:64], pr[3])

            pend_pop(99)
            # folds after the last q-conv chunk so the late qc copies are
            # never queued behind fold matmuls / diag dependencies
            for h in range(2):
                ftOO = fold_half([0, 1, 2, 3],
                                 ['v11', 'v10', 'v01', 'v00'], h)
                nc.scalar.copy(fscr[:, h * 2048:h * 2048 + 2048], ftOO[:])
            for h in range(2):
                ftOE = fold_half([4, 5], ['v10', 'v00'], h)
                nc.scalar.copy(pr[1][:, h * 2048:h * 2048 + 2048], ftOE[:])
            exp_group(6, 9)

            # ---- normalization ----
            nc.vector.tensor_reduce(sm[:, 27:28], sm[:, 9:18],
                                    axis=AX, op=ADD)
            nc.vector.reciprocal(sm[:, 28:29], sm[:, 27:28])
            r = sm[:, 28:29]

            # eo fold runs post-r: bake r into its diags (A = E*r)
            diag(6, rmul=True)
            diag(7, rmul=True)
            for h in range(2):
                ftEO = fold_half([6, 7], ['v01', 'v00'], h)
                nc.scalar.copy(pr[0][:, h * 2048:h * 2048 + 2048], ftEO[:])

            # ---- tail: finish planes in readiness order ----
            nc.vector.tensor_tensor(
                out=oOO.rearrange("p (r q) -> p r q", q=64),
                in0=fscr.rearrange("p (r q) -> p r q", q=64),
                in1=sEE[:, 1:65, 2:66], op=MULT)
            nc.scalar.activation(oOO[:], oOO[:], Copy, bias=0.0, scale=r)
            nc.sync.dma_start(out_d[:, 12288:16384], oOO[:])
            nc.vector.tensor_tensor(
                out=oOE.rearrange("p (r q) -> p r q", q=64),
                in0=pr[1].rearrange("p (r q) -> p r q", q=64),
                in1=sEO[:, 1:65, 0:64], op=MULT)
            nc.scalar.activation(oOE[:], oOE[:], Copy, bias=0.0, scale=r)
            nc.sync.dma_start(out_d[:, 8192:12288], oOE[:])
            # ee: (E4*r*v00) . sOO
            nc.vector.tensor_scalar(
                out=pr[3][:, 0:4096].rearrange("p (r q) -> p r q", q=64),
                in0=views['v00'], scalar1=sm[:, 17:18], scalar2=r,
                op0=MULT, op1=MULT)
            for h in range(2):
                nc.vector.tensor_tensor(
                    out=oEE[:, h * 2048:h * 2048 + 2048]
                        .rearrange("p (r q) -> p r q", q=64),
                    in0=pr[3][:, h * 2048:h * 2048 + 2048]
                        .rearrange("p (r q) -> p r q", q=64),
                    in1=sOO[:, h * 32:h * 32 + 32, 0:64], op=MULT)
                nc.sync.dma_start(out_d[:, h * 2048:h * 2048 + 2048],
                                  oEE[:, h * 2048:h * 2048 + 2048])
            for h in range(2):
                nc.vector.tensor_tensor(
                    out=oEO[:, h * 2048:h * 2048 + 2048]
                        .rearrange("p (r q) -> p r q", q=64),
                    in0=pr[0][:, h * 2048:h * 2048 + 2048]
                        .rearrange("p (r q) -> p r q", q=64),
                    in1=sOE[:, h * 32:h * 32 + 32, 2:66], op=MULT)
                nc.sync.dma_start(out_d[:, 4096 + h * 2048:4096 + (h + 1) * 2048],
                                  oEO[:, h * 2048:h * 2048 + 2048])

    nc.compile()
    return nc


def _get_program(add_bv: bool, per_tap_bias: bool):
    key = (add_bv, per_tap_bias)
    if key not in _prog_cache:
        _prog_cache[key] = _build(add_bv, per_tap_bias)
    return _prog_cache[key]


def _polyphase(x):
    B, C = x.shape[:2]
    ee = np.zeros((B, C, 65, 66), np.float16)
    ee[:, :, :, 1:66] = x[:, :, 0::2, 0::2]
    oe = np.zeros((B, C, 64, 66), np.float16)
    oe[:, :, :, 1:66] = x[:, :, 1::2, 0::2]
    return np.concatenate([
        ee.reshape(B, C, -1),
        x[:, :, 0::2, 1::2].reshape(B, C, -1),
        oe.reshape(B, C, -1),
        x[:, :, 1::2, 1::2].reshape(B, C, -1),
    ], axis=2)


def kernel(feat, src, Wq, bq, Wv, bv):
    from concourse.bass_utils import run_bass_kernel_spmd

    feat = np.asarray(feat, dtype=np.float32)
    src = np.asarray(src, dtype=np.float32)
    Wq = np.asarray(Wq, dtype=np.float32)
    bq = np.asarray(bq, dtype=np.float32)
    Wv = np.asarray(Wv, dtype=np.float32)
    bv = np.asarray(bv, dtype=np.float32)
    B, C, H, W = src.shape
    CH_HALF = C // 2

    src_pad = np.zeros((B, C, 129, 129), np.float16)
    src_pad[:, :, 1:129, 1:129] = src
    src_pk = _polyphase(src_pad)
    feat_pk = np.zeros((B, C, 65, 66), np.float16)
    feat_pk[:, :, 0:64, 0:64] = feat
    feat_pk = feat_pk.reshape(B, C, FEATN)

    add_bv = bool(np.any(bv))
    per_tap_bias = bool(np.any(bq))
    nc = _get_program(add_bv, per_tap_bias)
    ident = np.eye(128, dtype=np.float16)

    in_maps = []
    for core in range(N_CORES):
        b, u = core // 2, core % 2
        own = slice(CH_HALF * u, CH_HALF * u + CH_HALF)
        perm = np.r_[own, slice(CH_HALF * (1 - u), CH_HALF * (1 - u) + CH_HALF)]
        wpack = np.concatenate(
            [Wq[own][:, perm].T, Wv[own][:, perm].T], axis=1
        ).astype(np.float16)
        if per_tap_bias:
            k = feat[b, own].astype(np.float64)
            tot = k.sum((1, 2))
            no_r0 = tot - k[:, 0, :].sum(1)
            no_c0 = tot - k[:, :, 0].sum(1)
            no_rc = no_r0 - k[:, :, 0].sum(1) + k[:, 0, 0]
            sums = [no_rc, no_r0, no_r0, no_c0, tot, tot, no_c0, tot, tot]
            sinit_t = (np.stack(sums, 1) * bq[own, None] * SCALE).astype(
                np.float32)
            sinit = sinit_t[:, SLOT_ORDER]
        else:
            sinit = np.zeros((CH_HALF, 9), np.float32)
        in_maps.append({
            "src": np.ascontiguousarray(src_pk[b, perm]),
            "feat": np.ascontiguousarray(feat_pk[b, perm]),
            "wpack": np.ascontiguousarray(wpack),
            "s_init": sinit,
            "bv": bv[own].reshape(CH_HALF, 1).astype(np.float32),
            "ident": ident,
        })

    res = run_bass_kernel_spmd(nc, in_maps, list(range(N_CORES)),
                               trace=TRACE, **TRACE_KW)
    LAST_RESULT[0] = res

    out = np.empty((B, C, H, W), np.float32)
    for core in range(N_CORES):
        b, u = core // 2, core % 2
        own = slice(CH_HALF * u, CH_HALF * u + CH_HALF)
        r = res.results[core]["out"].astype(np.float32).reshape(
            CH_HALF, 4, 64, 64)
        out[b, own, 0::2, 0::2] = r[:, 0]
        out[b, own, 0::2, 1::2] = r[:, 1]
        out[b, own, 1::2, 0::2] = r[:, 2]
        out[b, own, 1::2, 1::2] = r[:, 3]
    return out


# revision 47
# speedup vs baseline: 1.1647x; 1.0600x over previous
"""Trainium2 Bass kernel for nn_CRAU (per-channel sparse attention).

Computation (per batch b, channel c):
  qc  = Wq @ src (1x1 conv)
  S[c,t] = sum_d unfold(qc)[c,t,d] * feat[c,d] * (1/64)      t in 3x3 window
  A   = softmax_t(S);  vc = Wv @ feat + bv
  out = fold(A outer vc) * src

Sharding: 8 cores = 4 batches x 2 output-channel halves (no collectives).

Final schedule (v7), built from trace-driven iteration:
 - qc chunks are plane-aligned <=2048 cols; ONE shared PSUM pool
   [128,2048]x2 rotates q-conv chunks, the v-conv, and the fold tiles,
   so the PE is never gated by more than one evacuation.
 - qc PSUM->SBUF f16 copies all on Scalar (its only bulk duty besides
   fold/v-conv evacuations) so the copy stream tracks the DMA.
 - ALL NINE taps run as Vector TTR custom ops (product+reduce, no
   Scalar accumulates); the EO and OO taps are split into row-aligned
   chunk halves so they start the moment their qc rows land.
 - folds run on TensorE as diag(E_t) matmuls (diag built on Vector
   from an uploaded identity), PSUM f32, evacuated by Scalar to f16;
   Vector does one full-plane TT (F~ . src) per parity plane.
 - 1/sumE is applied AFTER (F~ . src) as 4x-mode tensor_scalars; the
   eo plane (computed post-softmax anyway) bakes r into its diags, and
   the last two planes' TT+DMA are split in halves to start the
   output DMA earlier.
 - deferred Scalar ops carry an explicit readiness chunk index so
   emission order can never invert a cross-engine dependency.
Plane layouts as v1/v2 (polyphase packed padded 129x129 grid).
Accumulator slot order: [t0,t2,t6,t8, t1,t7, t3,t5, t4].
"""

import numpy as np

N_CORES = 8
SCALE = 1.0 / 64.0

P_EE, P_EO, P_OE, P_OO = 0, 4290, 8450, 12674
SRCN = 16770
FEATN = 4290
OUTN = 16384

_prog_cache = {}
TRACE = False
TRACE_KW = {}
LAST_RESULT = [None]

# plane-aligned chunks: EE 4290 | EO 4160 | OE 4224 | OO 4096
CHUNKS = [2048, 2048, 194, 2048, 2048, 64, 2048, 2048, 128, 2048, 2048]
EE_RDY, EO_RDY, OE_RDY = 2, 5, 8
SLOT = {0: 0, 2: 1, 6: 2, 8: 3, 1: 4, 7: 5, 3: 6, 5: 7, 4: 8}
SLOT_ORDER = [0, 2, 6, 8, 1, 7, 3, 5, 4]


def _build(add_bv: bool, per_tap_bias: bool):
    import concourse.mybir as mybir
    import concourse.tile as tile
    from concourse import bacc
    from concourse.dve_ops import TENSOR_TENSOR_REDUCE

    f32 = mybir.dt.float32
    f16 = mybir.dt.float16
    ADD = mybir.AluOpType.add
    MULT = mybir.AluOpType.mult
    AX = mybir.AxisListType.X
    Exp = mybir.ActivationFunctionType.Exp
    Copy = mybir.ActivationFunctionType.Copy

    nc = bacc.Bacc("TRN2", target_bir_lowering=False, debug=False,
                   num_devices=N_CORES)

    src_d = nc.dram_tensor("src", [256, SRCN], f16, kind="ExternalInput").ap()
    feat_d = nc.dram_tensor("feat", [256, FEATN], f16,
                            kind="ExternalInput").ap()
    wpack_d = nc.dram_tensor("wpack", [256, 256], f16,
                             kind="ExternalInput").ap()
    sinit_d = nc.dram_tensor("s_init", [128, 9], f32,
                             kind="ExternalInput").ap()
    bv_d = nc.dram_tensor("bv", [128, 1], f32, kind="ExternalInput").ap()
    ident_d = nc.dram_tensor("ident", [128, 128], f16,
                             kind="ExternalInput").ap()
    out_d = nc.dram_tensor("out", [128, OUTN], f16, kind="ExternalOutput").ap()

    coff = [0]
    for cs in CHUNKS:
        coff.append(coff[-1] + cs)

    with tile.TileContext(nc) as tc:
        with (
            tc.tile_pool(name="constp", bufs=2) as constp,
            tc.tile_pool(name="srcp", bufs=2) as srcp,
            tc.tile_pool(name="featp", bufs=2) as featp,
            tc.tile_pool(name="qcp", bufs=1) as qcp,
            tc.tile_pool(name="vcp", bufs=1) as vcp,
            tc.tile_pool(name="smp", bufs=1) as smp,
            tc.tile_pool(name="prodp", bufs=4) as prodp,
            tc.tile_pool(name="outp", bufs=4) as outp,
            tc.tile_pool(name="ps", bufs=2, space="PSUM") as ps,
        ):
            # smalls: [0:9] S by slot [9:18] E [52:60] EE halves [27] sumE
            # [28] r [32:41] s_init [48] bv
            sm = smp.tile([128, 64], f32, tag="smalls")
            nc.sync.dma_start(sm[:, 32:41], sinit_d[:, :])
            if add_bv:
                nc.sync.dma_start(sm[:, 48:49], bv_d[:, :])

            w_t = []
            for kt in range(2):
                wt = constp.tile([128, 256], f16, tag="w")
                nc.sync.dma_start(wt[:], wpack_d[128 * kt:128 * kt + 128, :])
                w_t.append(wt)
            ident = constp.tile([128, 128], f16, tag="ident")
            nc.sync.dma_start(ident[:], ident_d[:, :])

            feat_t = [featp.tile([128, FEATN], f16, tag="feat",
                                 name=f"feat{k}") for k in range(2)]

            src_t = [srcp.tile([128, SRCN], f16, tag="src", name=f"src{k}")
                     for k in range(2)]
            for c in range(len(CHUNKS)):
                if c == 3:
                    nc.sync.dma_start(feat_t[0][:], feat_d[0:128, :])
                if c == 5:
                    nc.sync.dma_start(feat_t[1][:], feat_d[128:256, :])
                for kt in range(2):
                    nc.sync.dma_start(
                        src_t[kt][:, coff[c]:coff[c + 1]],
                        src_d[128 * kt:128 * kt + 128, coff[c]:coff[c + 1]])

            qc = qcp.tile([128, SRCN], f16, tag="qc")
            qEE = qc[:, P_EE:P_EO].rearrange("p (r q) -> p r q", q=66)
            qEO = qc[:, P_EO:P_OE].rearrange("p (r q) -> p r q", q=64)
            qOE = qc[:, P_OE:P_OO].rearrange("p (r q) -> p r q", q=66)
            qOO = qc[:, P_OO:SRCN].rearrange("p (r q) -> p r q", q=64)
            kv = feat_t[0].rearrange("p (r q) -> p r q", q=66)[:, 0:64, 0:64]

            pr = [prodp.tile([128, 4096], f16, tag="prod", name=f"pr{k}")
                  for k in range(4)]
            dg = constp.tile([128, 9 * 128], f16, tag="diag")

            pend_s = []            # deferred (ready_chunk, scalar-op) items

            def pend_pop(c):
                done = [e for e in pend_s if e[0] <= c]
                for e in done:
                    e[1]()
                    pend_s.remove(e)

            def tap_ttr(t, qview, scr, sl=None, kview=None):
                sl = SLOT[t] if sl is None else sl
                kview = kv if kview is None else kview
                n = 1
                for d in qview.shape[1:]:
                    n *= d
                nc.vector._custom_dve(
                    TENSOR_TENSOR_REDUCE,
                    out=scr[:, 0:n].rearrange(
                        "p (r q) -> p r q", q=qview.shape[-1]),
                    in0=qview, in1=kview, s0=0.0,
                    s1=SCALE, accum_out=sm[:, sl:sl + 1])

            def tap_tth(qview, kview, scr, off, sl, ready):
                n = qview.shape[1] * qview.shape[2]
                s3 = scr[:, off:off + n].rearrange(
                    "p (r q) -> p r q", q=qview.shape[-1])
                nc.vector.tensor_tensor(out=s3, in0=qview, in1=kview,
                                        op=MULT)

                def acc(scr=scr, off=off, n=n, sl=sl):
                    nc.scalar.activation(scr[:, off:off + n],
                                         scr[:, off:off + n],
                                         Copy, bias=0.0, scale=SCALE,
                                         accum_out=sm[:, sl:sl + 1])
                pend_s.append((ready, acc))

            def exp_group(sl0, sl1):
                if per_tap_bias:
                    for sl in range(sl0, sl1):
                        nc.scalar.activation(sm[:, 9 + sl:10 + sl],
                                             sm[:, sl:sl + 1], Exp,
                                             bias=sm[:, 32 + sl:33 + sl],
                                             scale=1.0)
                else:
                    nc.scalar.activation(sm[:, 9 + sl0:9 + sl1],
                                         sm[:, sl0:sl1], Exp,
                                         bias=0.0, scale=1.0)

            def diag(sl, rmul=False):
                if rmul:
                    nc.vector.tensor_scalar(
                        out=dg[:, sl * 128:sl * 128 + 128], in0=ident[:],
                        scalar1=sm[:, 9 + sl:10 + sl],
                        scalar2=sm[:, 28:29], op0=MULT, op1=MULT)
                else:
                    nc.vector.tensor_scalar(
                        out=dg[:, sl * 128:sl * 128 + 128], in0=ident[:],
                        scalar1=sm[:, 9 + sl:10 + sl], scalar2=None,
                        op0=MULT)

            def emit_vconv():
                vc = vcp.tile([128, FEATN], f16, tag="vc")
                for c0 in (0, 2048, 4096):
                    csz = min(2048, FEATN - c0)
                    pt = ps.tile([128, 2048], f32, tag="mm")
                    for kt in range(2):
                        for s0 in range(0, csz, 512):
                            ssz = min(512, csz - s0)
                            nc.tensor.matmul(
                                pt[:, s0:s0 + ssz],
                                lhsT=w_t[kt][:, 128:256],
                                rhs=feat_t[kt][:, c0 + s0:c0 + s0 + ssz],
                                start=(kt == 0), stop=(kt == 1))
                    if add_bv:
                        nc.vector.tensor_scalar(
                            out=vc[:, c0:c0 + csz], in0=pt[:, 0:csz],
                            scalar1=sm[:, 48:49], scalar2=None, op0=ADD)
                    else:
                        nc.scalar.copy(vc[:, c0:c0 + csz], pt[:, 0:csz])
                vc3 = vc.rearrange("p (r q) -> p r q", q=66)
                if add_bv:
                    nc.gpsimd.memset(vc3[:, 64, :], 0.0)
                    nc.gpsimd.memset(vc3[:, :, 64:66], 0.0)
                return vc3

            vc3 = None
            views = {}

            s3 = src_t[0]
            sEE = s3[:, P_EE:P_EO].rearrange("p (r q) -> p r q", q=66)
            sEO = s3[:, P_EO:P_OE].rearrange("p (r q) -> p r q", q=64)
            sOE = s3[:, P_OE:P_OO].rearrange("p (r q) -> p r q", q=66)
            sOO = s3[:, P_OO:SRCN].rearrange("p (r q) -> p r q", q=64)

            oEE = outp.tile([128, 4096], f16, tag="O", name="oEE")
            oEO = outp.tile([128, 4096], f16, tag="O", name="oEO")
            oOE = outp.tile([128, 4096], f16, tag="O", name="oOE")
            oOO = outp.tile([128, 4096], f16, tag="O", name="oOO")

            fscr = pr[2]       # t8's product is consumed by chunk 7

            def fold_half(slots, vkeys, h, out_tile=None, srcv3=None):
                # diag matmuls, PSUM-accumulated 2048-col half. If
                # out_tile given: Vector TT multiplies PSUM F~ by the src
                # half directly; else returns PSUM tile for Scalar evac.
                ft = ps.tile([128, 2048], f32, tag="mm", name="ft")
                f3 = ft.rearrange("p (r q) -> p r q", q=64)
                for i, (sl, vk) in enumerate(zip(slots, vkeys)):
                    v3 = views[vk]
                    for b in range(4):
                        r0 = h * 32 + b * 8
                        nc.tensor.matmul(
                            f3[:, b * 8:b * 8 + 8, :],
                            lhsT=dg[:, sl * 128:sl * 128 + 128],
                            rhs=v3[:, r0:r0 + 8, :],
                            start=(i == 0), stop=(i == len(slots) - 1))
                if out_tile is None:
                    return ft
                nc.vector.tensor_tensor(
                    out=out_tile[:, h * 2048:h * 2048 + 2048]
                        .rearrange("p (r q) -> p r q", q=64),
                    in0=f3[:], in1=srcv3[:, h * 32:h * 32 + 32, :],
                    op=MULT)

            for c, csz in enumerate(CHUNKS):
                c0 = coff[c]
                if c == 7:
                    vc3 = emit_vconv()
                    views = dict(v00=vc3[:, 0:64, 0:64],
                                 v10=vc3[:, 1:65, 0:64],
                                 v01=vc3[:, 0:64, 1:65],
                                 v11=vc3[:, 1:65, 1:65])
                pt = ps.tile([128, 2048], f32, tag="mm")
                for kt in range(2):
                    for s0 in range(0, csz, 512):
                        ssz = min(512, csz - s0)
                        nc.tensor.matmul(
                            pt[:, s0:s0 + ssz],
                            lhsT=w_t[kt][:, 0:128],
                            rhs=src_t[kt][:, c0 + s0:c0 + s0 + ssz],
                            start=(kt == 0), stop=(kt == 1))
                nc.scalar.copy(qc[:, c0:c0 + csz], pt[:, 0:csz])
                pend_pop(c)

                if c == 0:
                    # EE tap first halves (rows fully inside chunk 0)
                    tap_ttr(0, qEE[:, 0:31, 1:65], pr[3], sl=52,
                            kview=kv[:, 0:31, :])
                    tap_tth(qEE[:, 0:31, 2:66], kv[:, 0:31, :],
                            pr[0], 0, 53, 2)
                    tap_tth(qEE[:, 1:31, 1:65], kv[:, 0:30, :],
                            pr[1], 0, 54, 2)
                    tap_ttr(8, qEE[:, 1:31, 2:66], pr[3], sl=55,
                            kview=kv[:, 0:30, :])
                if c == EE_RDY:
                    tap_ttr(0, qEE[:, 31:64, 1:65], pr[3], sl=56,
                            kview=kv[:, 31:64, :])
                    tap_tth(qEE[:, 31:64, 2:66], kv[:, 31:64, :],
                            pr[0], 1984, 57, 3)
                    tap_tth(qEE[:, 31:65, 1:65], kv[:, 30:64, :],
                            pr[1], 1920, 58, 4)
                    tap_ttr(8, qEE[:, 31:65, 2:66], pr[3], sl=59,
                            kview=kv[:, 30:64, :])

                    def fin_ee():
                        exp_group(0, 4)
                    pend_s.append((6, fin_ee))
                if c == 4:
                    tap_ttr(1, qEO[:, 0:64, 0:64], pr[3])
                if c == EO_RDY:
                    # EE half-sums (all 8 half accumulators written by now)
                    for i in range(4):
                        nc.vector.tensor_tensor(
                            out=sm[:, i:i + 1], in0=sm[:, 52 + i:53 + i],
                            in1=sm[:, 56 + i:57 + i], op=ADD)
                    tap_ttr(7, qEO[:, 1:65, 0:64], pr[3])

                    def fin_eo():
                        exp_group(4, 6)
                    pend_s.append((6, fin_eo))
                if c == OE_RDY:
                    for sl in range(6):
                        diag(sl)
                    tap_ttr(3, qOE[:, 0:64, 1:65], pr[3])
                    tap_ttr(5, qOE[:, 0:64, 2:66], pr[3])
                if c == 10:
                    tap_ttr(4, qOO[:, 0:64, 0:64], pr[3])

            pend_pop(99)
            # folds after the last q-conv chunk so the late qc copies are
            # never queued behind fold matmuls / diag dependencies
            for h in range(2):
                ftOO = fold_half([0, 1, 2, 3],
                                 ['v11', 'v10', 'v01', 'v00'], h)
                nc.scalar.copy(fscr[:, h * 2048:h * 2048 + 2048], ftOO[:])
            for h in range(2):
                ftOE = fold_half([4, 5], ['v10', 'v00'], h)
                nc.scalar.copy(pr[1][:, h * 2048:h * 2048 + 2048], ftOE[:])
            exp_group(6, 9)

            # ---- normalization ----
            nc.vector.tensor_reduce(sm[:, 27:28], sm[:, 9:18],
                                    axis=AX, op=ADD)
            nc.vector.reciprocal(sm[:, 28:29], sm[:, 27:28])
            r = sm[:, 28:29]

            # eo fold runs post-r: bake r into its diags (A = E*r)
            diag(6, rmul=True)
            diag(7, rmul=True)
            for h in range(2):
                ftEO = fold_half([6, 7], ['v01', 'v00'], h)
                nc.scalar.copy(pr[0][:, h * 2048:h * 2048 + 2048], ftEO[:])

            # ---- tail: finish planes in readiness order ----
            nc.vector.tensor_tensor(
                out=oOO.rearrange("p (r q) -> p r q", q=64),
                in0=fscr.rearrange("p (r q) -> p r q", q=64),
                in1=sEE[:, 1:65, 2:66], op=MULT)
            nc.vector.tensor_scalar(out=oOO[:], in0=oOO[:], scalar1=r,
                                    scalar2=None, op0=MULT)
            nc.sync.dma_start(out_d[:, 12288:16384], oOO[:])
            nc.vector.tensor_tensor(
                out=oOE.rearrange("p (r q) -> p r q", q=64),
                in0=pr[1].rearrange("p (r q) -> p r q", q=64),
                in1=sEO[:, 1:65, 0:64], op=MULT)
            nc.vector.tensor_scalar(out=oOE[:], in0=oOE[:], scalar1=r,
                                    scalar2=None, op0=MULT)
            nc.sync.dma_start(out_d[:, 8192:12288], oOE[:])
            # ee: (E4*r*v00) . sOO
            nc.vector.tensor_scalar(
                out=pr[3][:, 0:4096].rearrange("p (r q) -> p r q", q=64),
                in0=views['v00'], scalar1=sm[:, 17:18], scalar2=r,
                op0=MULT, op1=MULT)
            for h in range(2):
                nc.vector.tensor_tensor(
                    out=oEE[:, h * 2048:h * 2048 + 2048]
                        .rearrange("p (r q) -> p r q", q=64),
                    in0=pr[3][:, h * 2048:h * 2048 + 2048]
                        .rearrange("p (r q) -> p r q", q=64),
                    in1=sOO[:, h * 32:h * 32 + 32, 0:64], op=MULT)
                nc.sync.dma_start(out_d[:, h * 2048:h * 2048 + 2048],
                                  oEE[:, h * 2048:h * 2048 + 2048])
            for h in range(2):
                nc.vector.tensor_tensor(
                    out=oEO[:, h * 2048:h * 2048 + 2048]
                        .rearrange("p (r q) -> p r q", q=64),
                    in0=pr[0][:, h * 2048:h * 2048 + 2048]
                        .rearrange("p (r q) -> p r q", q=64),
                    in1=sOE[:, h * 32:h * 32 + 32, 2:66], op=MULT)
                nc.sync.dma_start(out_d[:, 4096 + h * 2048:4096 + (h + 1) * 2048],
                                  oEO[:, h * 2048:h * 2048 + 2048])

    nc.compile()
    return nc


def _get_program(add_bv: bool, per_tap_bias: bool):
    key = (add_bv, per_tap_bias)
    if key not in _prog_cache:
        _prog_cache[key] = _build(add_bv, per_tap_bias)
    return _prog_cache[key]


def _polyphase(x):
    B, C = x.shape[:2]
    ee = np.zeros((B, C, 65, 66), np.float16)
    ee[:, :, :, 1:66] = x[:, :, 0::2, 0::2]
    oe = np.zeros((B, C, 64, 66), np.float16)
    oe[:, :, :, 1:66] = x[:, :, 1::2, 0::2]
    return np.concatenate([
        ee.reshape(B, C, -1),
        x[:, :, 0::2, 1::2].reshape(B, C, -1),
        oe.reshape(B, C, -1),
        x[:, :, 1::2, 1::2].reshape(B, C, -1),
    ], axis=2)


def kernel(feat, src, Wq, bq, Wv, bv):
    from concourse.bass_utils import run_bass_kernel_spmd

    feat = np.asarray(feat, dtype=np.float32)
    src = np.asarray(src, dtype=np.float32)
    Wq = np.asarray(Wq, dtype=np.float32)
    bq = np.asarray(bq, dtype=np.float32)
    Wv = np.asarray(Wv, dtype=np.float32)
    bv = np.asarray(bv, dtype=np.float32)
    B, C, H, W = src.shape
    CH_HALF = C // 2

    src_pad = np.zeros((B, C, 129, 129), np.float16)
    src_pad[:, :, 1:129, 1:129] = src
    src_pk = _polyphase(src_pad)
    feat_pk = np.zeros((B, C, 65, 66), np.float16)
    feat_pk[:, :, 0:64, 0:64] = feat
    feat_pk = feat_pk.reshape(B, C, FEATN)

    add_bv = bool(np.any(bv))
    per_tap_bias = bool(np.any(bq))
    nc = _get_program(add_bv, per_tap_bias)
    ident = np.eye(128, dtype=np.float16)

    in_maps = []
    for core in range(N_CORES):
        b, u = core // 2, core % 2
        own = slice(CH_HALF * u, CH_HALF * u + CH_HALF)
        perm = np.r_[own, slice(CH_HALF * (1 - u), CH_HALF * (1 - u) + CH_HALF)]
        wpack = np.concatenate(
            [Wq[own][:, perm].T, Wv[own][:, perm].T], axis=1
        ).astype(np.float16)
        if per_tap_bias:
            k = feat[b, own].astype(np.float64)
            tot = k.sum((1, 2))
            no_r0 = tot - k[:, 0, :].sum(1)
            no_c0 = tot - k[:, :, 0].sum(1)
            no_rc = no_r0 - k[:, :, 0].sum(1) + k[:, 0, 0]
            sums = [no_rc, no_r0, no_r0, no_c0, tot, tot, no_c0, tot, tot]
            sinit_t = (np.stack(sums, 1) * bq[own, None] * SCALE).astype(
                np.float32)
            sinit = sinit_t[:, SLOT_ORDER]
        else:
            sinit = np.zeros((CH_HALF, 9), np.float32)
        in_maps.append({
            "src": np.ascontiguousarray(src_pk[b, perm]),
            "feat": np.ascontiguousarray(feat_pk[b, perm]),
            "wpack": np.ascontiguousarray(wpack),
            "s_init": sinit,
            "bv": bv[own].reshape(CH_HALF, 1).astype(np.float32),
            "ident": ident,
        })

    res = run_bass_kernel_spmd(nc, in_maps, list(range(N_CORES)),
                               trace=TRACE, **TRACE_KW)
    LAST_RESULT[0] = res

    out = np.empty((B, C, H, W), np.float32)
    for core in range(N_CORES):
        b, u = core // 2, core % 2
        own = slice(CH_HALF * u, CH_HALF * u + CH_HALF)
        r = res.results[core]["out"].astype(np.float32).reshape(
            CH_HALF, 4, 64, 64)
        out[b, own, 0::2, 0::2] = r[:, 0]
        out[b, own, 0::2, 1::2] = r[:, 1]
        out[b, own, 1::2, 0::2] = r[:, 2]
        out[b, own, 1::2, 1::2] = r[:, 3]
    return out
